# revision 22
# baseline (speedup 1.0000x reference)
"""Trainium2 Bass kernel for nn_AssignAttention (hard-assignment MoE-routing attention).

Math (forward): for each (b, h, key-token s), the key token is hard-assigned to
group n* = argmax_n (q_bhn . k_bhs); output per group = sum of assigned v vectors
scaled by 1/(count+1), then projected.  The straight-through softmax terms cancel
in forward up to ~1e-7, so only the argmax routing matters.

Strategy:
 - Pure data-parallel over batch B=16 across 8 cores (2 batches/core), no collectives.
 - Host precomputes t[b,h,n,:] = Wk_h^T Wq_h query[b,n] so attention logits are
   attn[s, (h,n)] = key[b,s,:] . t[b,h,n,:]  -- one C-contraction against raw key.
 - Host pre-transposes key to keyT [C, S]; all transfers use the (ct p) x ->
   p ct x rearrange, whose ~2KB-per-descriptor granularity measures fastest on
   the DMA queues (~23GB/s/queue; 6KB descriptors measured slower).
 - Attention logits use float32r matmuls (1 cyc/row, ~13-bit mantissa): measured
   argmax flip-induced error ~0.008 rel, well within tolerance. v/output paths in
   float32r/bf16.
 - Per 128-row s-subtile: argmax over each head's 64 logit columns (free-axis
   reduce_max + one broadcast is_equal -> bf16 one-hot on DVE), then
   PSUM-accumulate head-PAIR-packed o += aT_pair^T @ [v|1|v|1] (the ones column
   yields per-group counts; it is written once per rotating buffer, not per
   subtile).  The o-matmuls are flushed in one burst per CHUNK (after the next
   chunk's first subtile's attn/v): the PE pays its f32r<->bf16 reconfiguration
   penalty (~55ns) twice per burst instead of twice per subtile, and the extra
   pipeline depth keeps it off DVE's critical path.
 - Startup: the NEFF boot blocks all sequencers ~7.5us and each DMA trigger
   costs ~0.7us of sequencer time, so kt-chunk-0 plus batch 0's whole tc ride
   in ONE merged transfer (one trigger, one completion; 2KB rows), chunks ramp
   128/128/256/512..., and warmup matmuls absorb the PE pstate ramp while that
   transfer lands.
 - Epilogue scales by 1/(cnt+1) straight out of PSUM with two stride-0
   broadcast multiplies, transposes via PE, projects, DMAs out from the
   Activation queue (whose copy produced the data -- no cross-engine hop).
   During the last batch's epilogue a few scratch matmuls keep the PE clock
   from dropping out of its top p-state.
"""
import sys

sys.path.insert(0, "/opt/trn_rl_repo")

import numpy as np
import ml_dtypes

import concourse.bass as bass
import concourse.mybir as mybir
import concourse.tile as tile
from concourse.bass_utils import run_bass_kernel_spmd
from concourse.masks import make_identity

B, N, S, C, H = 16, 64, 4096, 384, 6
DH = C // H  # 64
NCORES = 8
BPC = B // NCORES  # batches per core = 2
CT = C // 128  # c-tiles = 3
# chunk boundaries: two tiny chunks and a half chunk so the DMA pipeline can
# feed the PE as soon as the merged first transfer lands, then 512-token chunks
CHUNK_BOUNDS = [0, 128, 256, 512] + list(range(1024, S, 512)) + [S]
CHUNKS = list(zip(CHUNK_BOUNDS[:-1], CHUNK_BOUNDS[1:]))

F32 = mybir.dt.float32
F32R = mybir.dt.float32r
BF16 = mybir.dt.bfloat16

LAST_RESULT = None  # stash of BassKernelResults for profiling in test.py


def _split_multiwaits(nc):
    """walrus codegen in this toolchain accepts at most one sync-wait per
    instruction; hoist extras onto standalone wait-only EventSemaphore
    instructions placed immediately before (same engine, so ordering holds)."""
    for fn in nc.m.functions:
        for blk in fn.blocks:
            new = []
            for inst in blk.instructions:
                si = inst.sync_info
                if si is not None and si.on_wait and len(si.on_wait) > 1:
                    for w in si.on_wait[:-1]:
                        ev = mybir.InstEventSemaphore(
                            name=nc.get_next_instruction_name(), ins=[], outs=[]
                        )
                        ev.engine = inst.engine
                        ev.sync_info = mybir.SyncInfo(on_wait=[w], on_update=[])
                        new.append(ev)
                    inst.sync_info = mybir.SyncInfo(
                        on_wait=[si.on_wait[-1]], on_update=si.on_update
                    )
                new.append(inst)
            blk.instructions = new


def _build_kernel():
    nc = bass.Bass()
    # pre: merged [kt chunk0 | tc] for batch 0; row (ct*128+p) = [key tokens
    # 0:128 | tc columns] of c-row ct*128+p, so each (p, ct) descriptor is 2KB
    pre_d = nc.declare_dram_parameter("pre", [C, 128 + C], F32R, isOutput=False)
    keyT_d = nc.declare_dram_parameter("keyT", [BPC, C, S], F32R, isOutput=False)
    tc_d = nc.declare_dram_parameter("tc", [BPC, C, C], F32R, isOutput=False)
    wvt_d = nc.declare_dram_parameter("wvt", [C, C], F32R, isOutput=False)
    wpt_d = nc.declare_dram_parameter("wpt", [C, C], BF16, isOutput=False)
    out_d = nc.declare_dram_parameter("out", [BPC, N, C], F32, isOutput=True)

    with tile.TileContext(nc) as tc:
        with (
            tc.tile_pool(name="consts", bufs=1) as consts,
            tc.tile_pool(name="perb", bufs=2) as perb,
            tc.tile_pool(name="keyp", bufs=6) as keyp,
            tc.tile_pool(name="work", bufs=1) as work,
            tc.tile_pool(name="epi", bufs=2) as epi,
            tc.tile_pool(name="ps_attn", bufs=3, space="PSUM") as ps_attn,
            tc.tile_pool(name="ps_v", bufs=2, space="PSUM") as ps_v,
            tc.tile_pool(name="ps_o", bufs=1, space="PSUM") as ps_o,
            tc.tile_pool(name="ps_epi", bufs=1, space="PSUM") as ps_epi,
        ):
            # one merged transfer delivers everything subtile 0 needs
            pre_sb = consts.tile([128, CT, 128 + C], F32R)
            nc.sync.dma_start(
                out=pre_sb[:],
                in_=pre_d.rearrange("(ct p) x -> p ct x", p=128),
            )
            kt_c0 = pre_sb[:, :, 0:128]
            tc_b0 = pre_sb[:, :, 128 : 128 + C]
            wvt_sb = consts.tile([128, CT, C], F32R)  # [c_in_p, ct, c_out]
            nc.sync.dma_start(
                out=wvt_sb[:], in_=wvt_d.rearrange("(ct p) co -> p ct co", p=128)
            )
            keyT_b0 = keyT_d[0].rearrange("(ct p) s -> p ct s", p=128)
            s0, s1 = CHUNKS[1]
            kt_c1 = keyp.tile([128, CT, s1 - s0], F32R, tag="kt")
            nc.sync.dma_start(out=kt_c1[:], in_=keyT_b0[:, :, s0:s1])
            s0, s1 = CHUNKS[2]
            kt_c2 = keyp.tile([128, CT, s1 - s0], F32R, tag="kt")
            nc.sync.dma_start(out=kt_c2[:], in_=keyT_b0[:, :, s0:s1])
            wpt_sb = consts.tile([128, CT, C], BF16)  # [hd_p, ct, c_out]
            nc.sync.dma_start(
                out=wpt_sb[:], in_=wpt_d.rearrange("(ct p) co -> p ct co", p=128)
            )
            # two stacked 64x64 identities so transposes of partition-offset-64
            # slices have a matching-base-partition rhs
            ident2 = consts.tile([128, N], BF16)
            make_identity(nc, ident2[0:N, :])
            make_identity(nc, ident2[N : 2 * N, :])

            # PE warmup: back-to-back matmuls on scratch while the first
            # transfer lands, so the pstate ramp completes before real work.
            # The psum bank is never read; its reuse starts with start=True.
            warm_sb = consts.tile([128, 640], BF16)
            nc.gpsimd.memset(warm_sb[:], 0.0)
            warm_ps = ps_attn.tile([128, 512], F32, tag="attn_ps")
            for _ in range(8):
                nc.tensor.matmul(
                    warm_ps[:], warm_sb[:, 0:128], warm_sb[:, 128:640],
                    start=True, stop=True,
                )

            # v65 ring: the ones column (counts) is written once per buffer;
            # the per-subtile copy only rewrites the v lanes
            v65_ring = [
                work.tile([128, H, DH + 1], BF16, tag=f"v65_{i}", name=f"v65_{i}")
                for i in range(8)
            ]
            for t in v65_ring:
                nc.gpsimd.memset(t[:, :, DH : DH + 1], 1.0)

            sub_ctr = 0
            for b in range(BPC):
                if b == 0:
                    tc_sb = tc_b0
                else:
                    tc_t = perb.tile([128, CT, C], F32R, tag="tc_sb")
                    nc.sync.dma_start(
                        out=tc_t[:],
                        in_=tc_d[b].rearrange("(ct p) hn -> p ct hn", p=128),
                    )
                    tc_sb = tc_t[:, :, :]
                # per-group accumulator, head-PAIR packed: for pair p, partition
                # rows 0..63 = head 2p groups, rows 64..127 = head 2p+1 groups;
                # col 64 = counts for both heads; cols 0..63 / 65..128 hold the
                # two heads' v-sums (off-diagonal blocks are junk, never read).
                # Zeroed explicitly; the accumulating matmuls use start=False so
                # their order doesn't matter (add-or-overwrite onto zeros commutes).
                o_ps = ps_o.tile([128, CT, 2 * DH + 2], F32)
                nc.vector.memset(o_ps[:], 0.0)

                keyT_b = keyT_d[b].rearrange("(ct p) s -> p ct s", p=128)
                # o-matmuls are flushed one chunk at a time, after the NEXT
                # chunk's first subtile's attn/v (see module docstring)
                pending = []  # [(aT, v65), ...] of the previous chunk

                def flush_o(stop):
                    for i, (aT_p, v65_p) in enumerate(pending):
                        last_sub = i == len(pending) - 1
                        for p in range(CT):
                            nc.tensor.matmul(
                                o_ps[:, p, :],
                                aT_p[:].rearrange("q h n -> q (h n)")[
                                    :, 2 * p * N : (2 * p + 2) * N
                                ],
                                v65_p[:].rearrange("q h d -> q (h d)")[
                                    :, 2 * p * (DH + 1) : (2 * p + 2) * (DH + 1)
                                ],
                                start=False,
                                stop=stop and last_sub and p == CT - 1,
                                skip_group_check=True,
                            )
                    pending.clear()

                for ci, (s0, s1) in enumerate(CHUNKS):
                    if b == 0 and ci == 0:
                        kt_sb = kt_c0
                    elif b == 0 and ci == 1:
                        kt_sb = kt_c1[:, :, :]
                    elif b == 0 and ci == 2:
                        kt_sb = kt_c2[:, :, :]
                    else:
                        kt_t = keyp.tile([128, CT, s1 - s0], F32R, tag="kt")
                        nc.sync.dma_start(
                            out=kt_t[:], in_=keyT_b[:, :, s0:s1]
                        )
                        kt_sb = kt_t[:, :, :]
                    carry = []
                    for sub in range((s1 - s0) // 128):
                        sl = slice(sub * 128, (sub + 1) * 128)
                        attn_ps = ps_attn.tile([128, C], F32)
                        v_ps = ps_v.tile([128, C], F32)
                        # all attn matmuls first so the logit group closes
                        # ~3 matmuls earlier and DVE's argmax starts sooner
                        for ct in range(CT):
                            nc.tensor.matmul(
                                attn_ps[:],
                                kt_sb[:, ct, sl],
                                tc_sb[:, ct, :],
                                start=(ct == 0),
                                stop=(ct == CT - 1),
                            )
                        for ct in range(CT):
                            nc.tensor.matmul(
                                v_ps[:],
                                kt_sb[:, ct, sl],
                                wvt_sb[:, ct, :],
                                start=(ct == 0),
                                stop=(ct == CT - 1),
                            )
                        if sub == 0 and pending:
                            flush_o(stop=False)
                        # per-head argmax -> one-hot (bf16); both ops read
                        # PSUM so they must stay on DVE (GpSimd/Pool cannot
                        # access PSUM)
                        gmax = work.tile([128, H], F32, tag="gmax", bufs=4)
                        nc.vector.reduce_max(
                            out=gmax[:],
                            in_=attn_ps[:].rearrange("p (h n) -> p h n", h=H),
                            axis=mybir.AxisListType.X,
                        )
                        aT = work.tile([128, H, N], BF16, tag="aT", bufs=8)
                        g = gmax[:]
                        g_bcast = bass.AP(
                            tensor=g.tensor, offset=g.offset,
                            ap=[g.ap[0], g.ap[1], [0, N]],
                        )
                        nc.vector.tensor_tensor(
                            out=aT[:],
                            in0=attn_ps[:].rearrange("p (h n) -> p h n", h=H),
                            in1=g_bcast,
                            op=mybir.AluOpType.is_equal,
                        )
                        # v lanes (bf16); the ones column is already in place
                        v65 = v65_ring[sub_ctr % 8]
                        sub_ctr += 1
                        nc.scalar.copy(
                            out=v65[:, :, 0:DH],
                            in_=v_ps[:].rearrange("p (h d) -> p h d", h=H),
                        )
                        carry.append((aT, v65))
                    pending.extend(carry)
                last_aT = pending[-1][0]
                flush_o(stop=True)
                if b == BPC - 1:
                    # keep the PE in its top p-state through the final
                    # epilogue (it drops after ~0.5us idle, which would slow
                    # the transposes/projection on the critical tail).  The
                    # scratch matmuls read the last subtile's one-hot so the
                    # scheduler cannot hoist them earlier.
                    warm_w = last_aT[:].rearrange("q h n -> q (h n)")[:, 0:128]
                    for _ in range(3):
                        nc.tensor.matmul(
                            warm_ps[:], warm_w, warm_sb[:, 128:640],
                            start=True, stop=True,
                        )
                # epilogue for this b: scale by 1/(cnt+1) (cnt in col 64 for
                # both heads of each pair) straight out of PSUM -- two
                # stride-0-broadcast multiplies -- then transpose to [hd, n],
                # project, and DMA out
                scl = epi.tile([128, CT], F32)
                nc.vector.tensor_scalar(
                    out=scl[:],
                    in0=o_ps[:, :, DH],
                    scalar1=1.0,
                    scalar2=None,
                    op0=mybir.AluOpType.add,
                )
                nc.vector.reciprocal(out=scl[:], in_=scl[:])
                osc = epi.tile([128, CT, DH], BF16)
                s0_ = scl[0:N, :]
                s0b = bass.AP(
                    tensor=s0_.tensor, offset=s0_.offset,
                    ap=[s0_.ap[0], s0_.ap[1], [0, DH]],
                )
                nc.vector.tensor_tensor(
                    out=osc[0:N, :, :],
                    in0=o_ps[0:N, :, 0:DH],
                    in1=s0b,
                    op=mybir.AluOpType.mult,
                )
                s1_ = scl[N : 2 * N, :]
                s1b = bass.AP(
                    tensor=s1_.tensor, offset=s1_.offset,
                    ap=[s1_.ap[0], s1_.ap[1], [0, DH]],
                )
                nc.vector.tensor_tensor(
                    out=osc[N : 2 * N, :, :],
                    in0=o_ps[N : 2 * N, :, DH + 1 : 2 * DH + 1],
                    in1=s1b,
                    op=mybir.AluOpType.mult,
                )
                # osc[0:64, p, :] = [n, dh] of head 2p -> oT rows 128p+dh;
                # osc[64:128, p, :] = [n, dh] of head 2p+1 -> oT rows 128p+64+dh
                oT_ps = ps_epi.tile([128, CT, N], BF16)
                for p in range(CT):
                    nc.tensor.transpose(
                        oT_ps[0:N, p, :], osc[0:N, p, :], ident2[0:N, :]
                    )
                    nc.tensor.transpose(
                        oT_ps[N : 2 * N, p, :],
                        osc[N : 2 * N, p, :],
                        ident2[N : 2 * N, :],
                    )
                oT_sb = epi.tile([128, CT, N], BF16)
                nc.scalar.copy(out=oT_sb[0:N], in_=oT_ps[0:N])
                nc.vector.tensor_copy(out=oT_sb[N : 2 * N], in_=oT_ps[N : 2 * N])
                out_ps = ps_epi.tile([N, C], F32)
                for ct in range(CT):
                    nc.tensor.matmul(
                        out_ps[:],
                        oT_sb[:, ct, :],
                        wpt_sb[:, ct, :],
                        start=(ct == 0),
                        stop=(ct == CT - 1),
                    )
                out_sb = epi.tile([N, C], F32)
                nc.scalar.copy(out=out_sb[0 : N // 2], in_=out_ps[0 : N // 2])
                nc.vector.tensor_copy(
                    out=out_sb[N // 2 : N], in_=out_ps[N // 2 : N]
                )
                # trigger from the Sync queue: its end-of-kernel drain block
                # is the shortest, so the final barrier is met soonest
                nc.sync.dma_start(out=out_d[b], in_=out_sb[:])

    _split_multiwaits(nc)
    return nc


_NC_CACHE = None


def _get_nc():
    global _NC_CACHE
    if _NC_CACHE is None:
        _NC_CACHE = _build_kernel()
    return _NC_CACHE


def kernel(query, key, Wq, Wk, Wv, Wp, bp):
    global LAST_RESULT
    query = np.ascontiguousarray(query, dtype=np.float32)
    key = np.ascontiguousarray(key, dtype=np.float32)
    Wq = np.asarray(Wq, dtype=np.float32)
    Wk = np.asarray(Wk, dtype=np.float32)
    Wv = np.asarray(Wv, dtype=np.float32)
    Wp = np.asarray(Wp, dtype=np.float32)
    bp = np.asarray(bp, dtype=np.float32)

    # host prep: t[b,h,n,:] = Wk_h^T Wq_h query[b,n]  (tiny; never touches `key`)
    q = query @ Wq.T  # [B, N, C]
    qh = q.reshape(B, N, H, DH).transpose(0, 2, 1, 3)  # [B,H,N,DH]
    Wk_h = Wk.reshape(H, DH, C)
    t = np.einsum("bhnd,hdc->bhnc", qh, Wk_h)  # [B,H,N,C]
    # Tc[b] layout: [C, (h n)] with column h*N+n = t[b,h,n,:]
    Tc = np.ascontiguousarray(
        t.transpose(0, 3, 1, 2).reshape(B, C, H * N), dtype=np.float32
    )
    keyT = np.ascontiguousarray(key.transpose(0, 2, 1), dtype=np.float32)  # [B,C,S]
    WvT = np.ascontiguousarray(Wv.T, dtype=np.float32)
    WpT = np.ascontiguousarray(Wp.T).astype(ml_dtypes.bfloat16)
    # merged first transfer per core (batch 0 of that core): [kt chunk0 | tc]
    pre_all = [
        np.ascontiguousarray(
            np.concatenate([keyT[i * BPC][:, 0:128], Tc[i * BPC]], axis=1)
        )
        for i in range(NCORES)
    ]

    nc = _get_nc()
    in_maps = [
        {
            "pre": pre_all[i],
            "keyT": keyT[i * BPC : (i + 1) * BPC],
            "tc": Tc[i * BPC : (i + 1) * BPC],
            "wvt": WvT,
            "wpt": WpT,
        }
        for i in range(NCORES)
    ]
    try:
        res = run_bass_kernel_spmd(nc, in_maps, core_ids=list(range(NCORES)))
    except Exception:
        # transient NRT device errors have been observed; retry once
        res = run_bass_kernel_spmd(nc, in_maps, core_ids=list(range(NCORES)))
    LAST_RESULT = res
    out = np.concatenate([res.results[i]["out"] for i in range(NCORES)], axis=0)
    return (out + bp).astype(np.float32)
